# revision 1
# baseline (speedup 1.0000x reference)
"""Trainium2 Bass kernel for nn_Conv_LI (leaky-integrator + 5x5 'same' conv).

Math: with the reference constants, DT*TAU_MEM_INV = 1.0, so the LI cell
collapses to
    vs[t] = i_t,   i_{t+1} = (i_t - 0.2*i_t) + x_t,   i_0 = 0
(an exponential moving accumulation over time), followed by a per-timestep
5x5 cross-correlation with 'same' zero padding.

Distribution: H is sharded across the 8 cores (64 output rows each). Each
core receives its 64 rows plus a 2-row halo on each side (zero-padded at the
global edges), so no inter-core communication is needed.

Per-core pipeline (all 8 cores run the same program, SPMD):
  - x arrives host-side time-shifted by one (vs[t] needs x[t-1]) and
    zero-padded to [T, 68, 516] (h halo + w halo).
  - DMA a window of TW=16 timesteps into SBUF, layout [68 part, t, w].
  - EMA on VectorE: one scalar_tensor_tensor per timestep:
        vs[s] = (vs[s-1] * 0.8) + x[s]
  - 5x5 conv on TensorE as 5 PSUM-accumulated banded matmuls (contraction
    over the h-halo partitions; the dx shifts are free-dim AP offsets).
    Two timesteps are column-packed per PSUM tile (tile_position (0,0) and
    (0,64)) so the full 128 PE columns are used.
  - ScalarE copies PSUM -> SBUF, then DMA out.
"""

import numpy as np

T_FULL, H_FULL, W_FULL = 256, 512, 512
N_CORES = 8
HC = H_FULL // N_CORES  # 64 output rows per core
HP = HC + 4             # 68 partition rows incl 2+2 halo
WP = W_FULL + 4         # 516 padded width
TW = 16                 # timesteps per window
DECAY = 0.8

_PROG_CACHE = {}


def _build_program(t_total):
    import concourse.bacc as bacc
    import concourse.mybir as mybir
    import concourse.tile as tile

    f32 = mybir.dt.float32
    f32r = mybir.dt.float32r
    mult = mybir.AluOpType.mult
    add = mybir.AluOpType.add

    assert t_total % TW == 0
    nwin = t_total // TW

    nc = bacc.Bacc(None, target_bir_lowering=False)
    x = nc.dram_tensor("x", [t_total, HP, WP], f32, kind="ExternalInput")
    lw_d = nc.dram_tensor("lw", [HP, 5 * HC], f32r, kind="ExternalInput")
    out = nc.dram_tensor("out", [t_total, HC, W_FULL], f32, kind="ExternalOutput")

    with tile.TileContext(nc) as tc:
        with (
            tc.tile_pool(name="const", bufs=1) as cpool,
            tc.tile_pool(name="xw", bufs=2) as xpool,
            tc.tile_pool(name="vs", bufs=2) as vpool,
            tc.tile_pool(name="ob", bufs=4) as opool,
            tc.tile_pool(name="ps", bufs=4, space="PSUM") as ppool,
        ):
            lw = cpool.tile([HP, 5 * HC], f32r)
            nc.sync.dma_start(out=lw[:HP, :], in_=lw_d[:, :])
            zt = cpool.tile([HP, WP], f32)
            nc.vector.memset(zt[:HP, :], 0.0)

            prev = None
            for win in range(nwin):
                t0 = win * TW
                xw = xpool.tile([HP, TW * WP], f32)
                nc.sync.dma_start(
                    out=xw[:HP, :].rearrange("h (t w) -> h t w", t=TW),
                    in_=x[t0 : t0 + TW].rearrange("t h w -> h t w"),
                )
                vs = vpool.tile([HP, TW * WP], f32r)
                # Wait-absorbing fence: scalar_tensor_tensor's ISA struct only
                # supports a single sync wait, so soak up the DMA-completion
                # and vs-slot-reuse waits on a cheap copy first.
                nc.vector.tensor_copy(out=vs[:HP, 0:4], in_=xw[:HP, 0:4])
                for s in range(TW):
                    cur = vs[:HP, s * WP : (s + 1) * WP]
                    p = zt[:HP, :] if prev is None else prev
                    nc.vector.scalar_tensor_tensor(
                        out=cur,
                        in0=p,
                        scalar=DECAY,
                        in1=xw[:HP, s * WP : (s + 1) * WP],
                        op0=mult,
                        op1=add,
                    )
                    prev = cur
                for pr in range(TW // 2):
                    sa = 2 * pr
                    # two timesteps share one 2-bank PSUM tile (free halves)
                    ps = ppool.tile([HC, 2 * W_FULL], f32)
                    for half in range(2):
                        s = sa + half
                        for dx in range(5):
                            lwx = lw[:HP, dx * HC : (dx + 1) * HC]
                            nc.tensor.matmul(
                                ps[0:HC, half * W_FULL : (half + 1) * W_FULL],
                                lwx,
                                vs[:HP, s * WP + dx : s * WP + dx + W_FULL],
                                start=(dx == 0),
                                stop=(dx == 4),
                            )
                    ob = opool.tile([HC, 2 * W_FULL], f32)
                    nc.scalar.copy(out=ob[0:HC, :], in_=ps[0:HC, :])
                    nc.sync.dma_start(
                        out=out[t0 + sa : t0 + sa + 2].rearrange("t h w -> h t w"),
                        in_=ob[0:HC, :].rearrange("h (t w) -> h t w", t=2),
                    )
    nc.finalize()
    return nc


def _get_program(t_total):
    if t_total not in _PROG_CACHE:
        _PROG_CACHE[t_total] = _build_program(t_total)
    return _PROG_CACHE[t_total]


def _host_prep(x, k, t_total):
    """Build per-core shifted+padded inputs and the banded lhsT matrices."""
    x = np.asarray(x, dtype=np.float32)
    k = np.asarray(k, dtype=np.float32)
    # time-shift by one (vs[t] = EMA consumes x[t-1]) and zero-pad h/w by 2
    xs = np.zeros((t_total, H_FULL + 4, W_FULL + 4), np.float32)
    xs[1:, 2 : H_FULL + 2, 2 : W_FULL + 2] = x[: t_total - 1, 0]
    # banded conv matrices: lhsT[p, dx, j] = k[p - j, dx] for p - j in [0, 5)
    lwh = np.zeros((HP, 5, HC), np.float32)
    j = np.arange(HC)
    for dy in range(5):
        for dx in range(5):
            lwh[j + dy, dx, j] = k[dy, dx]
    lwh = np.ascontiguousarray(lwh.reshape(HP, 5 * HC))
    in_maps = []
    for c in range(N_CORES):
        xc = np.ascontiguousarray(xs[:, c * HC : c * HC + HP, :])
        in_maps.append({"x": xc, "lw": lwh})
    return in_maps


def kernel(x, kernel):
    from concourse.bass_utils import run_bass_kernel_spmd

    t_total = x.shape[0]
    in_maps = _host_prep(x, kernel, t_total)
    nc = _get_program(t_total)
    res = run_bass_kernel_spmd(nc, in_maps, list(range(N_CORES)))
    out = np.empty((t_total, 1, H_FULL, W_FULL), np.float32)
    for c in range(N_CORES):
        out[:, 0, c * HC : (c + 1) * HC, :] = np.asarray(res.results[c]["out"])
    return out



# revision 3
# speedup vs baseline: 2.6875x; 2.6875x over previous
"""Trainium2 Bass kernel for nn_Conv_LI (leaky-integrator + 5x5 'same' conv).

Math: with the reference constants DT*TAU_MEM_INV = 1.0, the LI cell collapses
to vs[t] = i_t with i_{t+1} = 0.8*i_t + x_t, i_0 = 0 (EMA over time), followed
by a per-timestep 5x5 cross-correlation with 'same' zero padding.

Distribution (8 cores = 4 H-bands x 2 T-halves): each core owns 128 output
rows (H) and 128 timesteps. The EMA recurrence is made shardable in T by a
24-step warmup (0.8^24 ~ 5e-3, far below the tolerance).

Layout is transposed to [w, h]: W lives on SBUF partitions in 5 bands of
124 output columns (+4 halo = 128 contraction rows), H+halo (132) on the free
dim. The 5x5 conv then needs only 5 matmuls (one per dy, dx folded into the
banded contraction) per 124-wide band, each sweeping 512 free elements
(4 timesteps x 128 h). Everything runs in bf16 (tolerance is 2e-2).

Per-core pipeline:
  - window = 8 timesteps: DMA x in as [128 part, 5 band x 8 t x 132 h] bf16
    (one 10.5 KB contiguous chunk per partition line).
  - EMA on VectorE: one scalar_tensor_tensor per timestep over [128, 5x132].
  - conv on TensorE: psum tile [124, 4 banks]; each bank = one (4-timestep,
    band) unit = 5 accumulated banded matmuls (bf16, free dim 512).
  - ScalarE copies psum -> SBUF as bf16 (one [124, 2048] copy per tile) and
    triggers the output DMA (2 tiles per DMA) on its own HWDGE queue; input
    DMAs ride the Sync queue.
"""

import numpy as np

T_FULL, H_FULL, W_FULL = 256, 512, 512
N_CORES = 8
N_HB = 4          # H bands across cores
N_TH = 2          # T halves across cores
HB = 128          # output h rows per core
HBP = HB + 4      # 132 h extent incl halo
WARM = 24         # EMA warmup steps for T-half cores
NT_WIN = 8        # timesteps per DMA/EMA window
TLEN = T_FULL // N_TH + WARM            # 152 EMA steps per core
NWIN = TLEN // NT_WIN                   # 19 windows
WARM_WIN = WARM // NT_WIN               # 3 warmup windows
BW = 124          # output w columns per band
NB = 5            # w bands (4*124 + 16 = 512)
BW4 = W_FULL - 4 * BW                   # 16: width of the last band
NQ = 32           # 4-timestep quads per core (128 real timesteps)
N_UNITS = NQ * NB                       # 160 psum units per core
N_PTILES = N_UNITS // 4                 # 40 psum tiles (4 banks each)
WIN_ELEMS = NB * NT_WIN * HBP           # 5280 free elems per window tile

_PROG_CACHE = {}


def _build_program():
    import concourse.bacc as bacc
    import concourse.mybir as mybir
    import concourse.tile as tile

    bf16 = mybir.dt.bfloat16
    f32 = mybir.dt.float32
    mult = mybir.AluOpType.mult
    add = mybir.AluOpType.add

    nc = bacc.Bacc(None, target_bir_lowering=False)
    xc = nc.dram_tensor("xc", [NWIN, 128, WIN_ELEMS], bf16, kind="ExternalInput")
    lw_d = nc.dram_tensor("lw", [128, 2 * 5 * BW], bf16, kind="ExternalInput")
    od = nc.dram_tensor("od", [N_PTILES // 2, BW, 2 * 4 * 512], bf16,
                        kind="ExternalOutput")

    # psum tile k holds units 4k..4k+3; unit u = (quad g=u//5, band b=u%5);
    # quad g uses window 3 + g//2. Emit tile k right after its last window.
    tiles_by_win = {}
    for k in range(N_PTILES):
        wreq = WARM_WIN + ((4 * k + 3) // 5) // 2
        tiles_by_win.setdefault(wreq, []).append(k)

    with tile.TileContext(nc) as tc:
        with (
            tc.tile_pool(name="const", bufs=1) as cpool,
            tc.tile_pool(name="xw", bufs=2) as xpool,
            tc.tile_pool(name="vs", bufs=3) as vpool,
            tc.tile_pool(name="ob", bufs=3) as opool,
            tc.tile_pool(name="ps", bufs=2, space="PSUM") as ppool,
        ):
            lw = cpool.tile([128, 2 * 5 * BW], bf16)
            nc.sync.dma_start(out=lw[:128, :], in_=lw_d[:, :])
            zt = cpool.tile([128, NB * HBP], bf16)
            nc.vector.memset(zt[:128, :], 0.0)

            prev = None          # [p, (b, h)] slice of the previous timestep
            vstiles = []
            osb = None
            for win in range(NWIN):
            # --- input DMA + EMA for this window ---
                xw = xpool.tile([128, WIN_ELEMS], bf16)
                nc.sync.dma_start(out=xw[:128, :], in_=xc[win])
                vs = vpool.tile([128, WIN_ELEMS], bf16)
                vstiles.append(vs)
                # Wait-absorbing fence (stt supports a single sync wait).
                nc.vector.tensor_copy(out=vs[:128, 0:4], in_=xw[:128, 0:4])
                xw3 = xw[:128, :].rearrange("p (b t h) -> p b t h", b=NB, t=NT_WIN)
                vs3 = vs[:128, :].rearrange("p (b t h) -> p b t h", b=NB, t=NT_WIN)
                for s in range(NT_WIN):
                    cur = vs3[:, :, s, :]
                    p = zt[:128, :].rearrange("p (b h) -> p b h", b=NB) \
                        if prev is None else prev
                    nc.vector.scalar_tensor_tensor(
                        out=cur, in0=p, scalar=0.8, in1=xw3[:, :, s, :],
                        op0=mult, op1=add,
                    )
                    prev = cur

                # --- conv for psum tiles whose windows are now complete ---
                for k in tiles_by_win.get(win, []):
                    ps = ppool.tile([BW, 4 * 512], f32)
                    for q in range(4):
                        u = 4 * k + q
                        g, b = divmod(u, NB)
                        vsl = vstiles[WARM_WIN + g // 2]
                        th4 = (g % 2) * 4
                        npart = 128 if b < 4 else BW4 + 4
                        lwoff = 0 if b < 4 else 5 * BW
                        v3 = vsl[0:npart, :].rearrange(
                            "p (b t h) -> p b t h", b=NB, t=NT_WIN)
                        for dy in range(5):
                            nc.tensor.matmul(
                                ps[0:BW, q * 512:(q + 1) * 512],
                                lw[0:npart, lwoff + dy * BW: lwoff + (dy + 1) * BW],
                                v3[:, b, th4:th4 + 4, dy:dy + HB],
                                start=(dy == 0),
                                stop=(dy == 4),
                            )
                    if k % 2 == 0:
                        osb = opool.tile([BW, 2 * 4 * 512], bf16)
                    nc.scalar.copy(
                        out=osb[0:BW, (k % 2) * 2048:(k % 2) * 2048 + 2048],
                        in_=ps[0:BW, :],
                    )
                    if k % 2 == 1:
                        nc.scalar.dma_start(out=od[k // 2], in_=osb[0:BW, :])
    nc.finalize()
    return nc


def _get_program(t_total=T_FULL):
    if t_total not in _PROG_CACHE:
        _PROG_CACHE[t_total] = _build_program()
    return _PROG_CACHE[t_total]


def _host_prep(x, k, t_total=T_FULL):
    """Per-core [w, p, (band, t, h)] bf16 inputs + banded lhsT matrices."""
    import ml_dtypes

    x = np.asarray(x, dtype=np.float32)
    k = np.asarray(k, dtype=np.float32)
    # [t, w, h], EMA at step t consumes x[t-1]; pad w by 2 (+band slack), h by 2
    xT = x[:, 0].transpose(0, 2, 1)
    xbuf = np.zeros((N_TH * (T_FULL // N_TH) + WARM, 4 + NB * BW, H_FULL + 4),
                    np.float32)
    xbuf[WARM + 1: WARM + T_FULL, 2: 2 + W_FULL, 2: 2 + H_FULL] = xT[: T_FULL - 1]

    lwm = np.zeros((128, 2 * 5 * BW), np.float32)
    for dy in range(5):
        for dx in range(5):
            j = np.arange(BW)
            lwm[j + dx, dy * BW + j] = k[dy, dx]
            j4 = np.arange(BW4)
            lwm[j4 + dx, 5 * BW + dy * BW + j4] = k[dy, dx]
    lwm = lwm.astype(ml_dtypes.bfloat16)

    in_maps = []
    for c in range(N_CORES):
        th, hb = divmod(c, N_HB)
        xs = xbuf[th * (T_FULL // N_TH): th * (T_FULL // N_TH) + TLEN]
        bands = np.stack(
            [xs[:, BW * b: BW * b + 128, hb * HB: hb * HB + HBP]
             for b in range(NB)], axis=1)            # [152, 5, 128, 132]
        xcore = (bands.reshape(NWIN, NT_WIN, NB, 128, HBP)
                 .transpose(0, 3, 2, 1, 4)
                 .reshape(NWIN, 128, WIN_ELEMS)
                 .astype(ml_dtypes.bfloat16))
        in_maps.append({"xc": np.ascontiguousarray(xcore), "lw": lwm})
    return in_maps


def kernel(x, kernel):
    from concourse.bass_utils import run_bass_kernel_spmd

    t_total = x.shape[0]
    in_maps = _host_prep(x, kernel, t_total)
    nc = _get_program(t_total)
    res = run_bass_kernel_spmd(nc, in_maps, list(range(N_CORES)))
    out = np.empty((t_total, 1, H_FULL, W_FULL), np.float32)
    for c in range(N_CORES):
        th, hb = divmod(c, N_HB)
        a = np.asarray(res.results[c]["od"]).astype(np.float32)
        # [kk, p, half, q, t', h] -> u = 8kk+4half+q -> [g, b] = divmod(u, 5)
        a = a.reshape(N_PTILES // 2, BW, 2, 4, 4, HB).transpose(0, 2, 3, 4, 1, 5)
        a = a.reshape(NQ, NB, 4, BW, HB).transpose(0, 2, 4, 1, 3)
        a = a.reshape(NQ * 4, HB, NB * BW)[:, :, :W_FULL]
        out[th * (T_FULL // N_TH): (th + 1) * (T_FULL // N_TH), 0,
            hb * HB: (hb + 1) * HB, :] = a
    return out
